# revision 30
# baseline (speedup 1.0000x reference)
"""Causal single-head attention (B=4, T=4096, E=204, H=64) on 8 NeuronCores.

Sharding: data-parallel over batch (2 cores per batch element); each core
handles the interleaved half of the 128-row query tiles of its batch. For
odd-parity cores the host swaps adjacent 128-row tile pairs of x so that the
causal loop structure (extents + masks) is identical across all 8 cores -->
one SPMD program, balanced work.

v2: the kernel is restructured around the ACT (scalar) engine, whose ~35us of
exp work is the hard floor. Differences vs v1 (78us):
  * Projections are no longer a monolithic prolog: only K tile 0 and Q chunks
    0/1 are computed up front; the remaining K/V/Q projection work is emitted
    as small "units" woven between attention iterations, filling PE slack.
    First exp issues at ~4us instead of ~20us.
  * The scalar queue carries ONLY exps: DMA triggers moved to sync/gpsimd/
    vector, V-cast copies to DVE, and the exp table is preloaded via a dummy
    activation at t=0 so the implicit ACT_TABLE_LOAD (1.3us) is off the
    critical path.
  * Q is projected only for the core's own 2048 queries (strided moving AP
    over the even 128-col tiles) -- halves Q projection cost.
  * Single-accumulator iterations write both k-tiles' scores compactly
    (no dead zone), cutting ~1.5k wasted exp columns.
  * Epilogues run inline (deferred 2 iterations so the PE never waits on the
    DVE oa-copy), not in a post-loop tail: v1 lost ~15us to a tail running at
    the demoted 1.2GHz clock. A short warm burst after the last AV keeps the
    clock up through the final epilogue.
PSUM budget: S ring 2x2 banks + unit ring 2x1 + accumulators 2x1 = 8 banks.
"""
import sys

if "/opt/trn_rl_repo" not in sys.path:
    sys.path.insert(0, "/opt/trn_rl_repo")

import numpy as np

B, T, E, H = 4, 4096, 204, 64
E1 = 128
E2 = E - E1  # 76
NT = T // 128  # 32 k-tiles
NCORES = 8
SCALE = 1.0 / float(np.sqrt(E))

_CACHE = {}


def _build_nc():
    from contextlib import ExitStack

    import concourse.bacc as bacc
    import concourse.bass as bass
    import concourse.mybir as mybir
    import concourse.tile as tile
    from concourse.masks import make_identity

    f32 = mybir.dt.float32
    bf16 = mybir.dt.bfloat16
    Exp = mybir.ActivationFunctionType.Exp

    nc = bacc.Bacc("TRN2", target_bir_lowering=False, debug=False)

    # host supplies x^T (pair-swapped for odd cores), bf16
    xta_d = nc.dram_tensor("xta", [E1, T], bf16, kind="ExternalInput")
    xtb_d = nc.dram_tensor("xtb", [E2, T], bf16, kind="ExternalInput")
    # host-packed even tiles of x cols 0:2048 = this core's Q chunks 0+1
    xqa_d = nc.dram_tensor("xqa", [E1, 1024], bf16, kind="ExternalInput")
    xqb_d = nc.dram_tensor("xqb", [E2, 1024], bf16, kind="ExternalInput")
    w_d = nc.dram_tensor("w_all", [E, 3 * H], bf16, kind="ExternalInput")
    # 1.0 = keep, 0.0 = mask, applied to the block past the diagonal tile
    pad_d = nc.dram_tensor("pad01", [128, 1], f32, kind="ExternalInput")
    y_d = nc.dram_tensor("y", [T // 2, H], f32, kind="ExternalOutput")

    with tile.TileContext(nc) as tc, ExitStack() as ctx:
        const = ctx.enter_context(tc.tile_pool(name="const", bufs=1))
        big = ctx.enter_context(tc.tile_pool(name="big", bufs=1))
        ppool = ctx.enter_context(tc.tile_pool(name="pp", bufs=4))
        oapool = ctx.enter_context(tc.tile_pool(name="oa", bufs=3))
        rpool = ctx.enter_context(tc.tile_pool(name="rp", bufs=2))
        spool = ctx.enter_context(
            tc.tile_pool(name="S", bufs=2, space=bass.MemorySpace.PSUM)
        )
        upool = ctx.enter_context(
            tc.tile_pool(name="U", bufs=2, space=bass.MemorySpace.PSUM)
        )
        accpool = ctx.enter_context(
            tc.tile_pool(name="acc", bufs=1, space=bass.MemorySpace.PSUM)
        )

        # ---- const tiles; identb first so the PE warm burst can start early
        identb = const.tile([128, 128], bf16)
        identf = const.tile([128, 128], f32)
        tri01 = const.tile([128, 128], bf16)
        make_identity(nc, identb[:])  # gpsimd

        w_a = const.tile([E1, 3 * H], bf16)
        w_b = const.tile([E2, 3 * H], bf16)
        pad_sb = const.tile([128, 1], f32)
        xT_a = big.tile([E1, T], bf16)
        xT_b = big.tile([E2, T], bf16)
        xq_a = big.tile([E1, 1024], bf16)
        xq_b = big.tile([E2, 1024], bf16)

        # ---- DMAs. A transfer is descriptor-bound (~1 descriptor per
        # partition row); the sync and gpsimd trigger queues map to 16-engine
        # DMA queues, scalar's to a 4-engine one. All prolog-critical data
        # (x cols 0:2048, w) goes on sync/gpsimd split by partition halves;
        # the second half of x rides the slow scalar queue (needed ~20us in).
        nc.sync.dma_start(xq_a[0:64, :], xqa_d[0:64, :])
        nc.gpsimd.dma_start(xq_a[64:E1, :], xqa_d[64:E1, :])
        nc.sync.dma_start(xT_a[0:64, 0:256], xta_d[0:64, 0:256])
        nc.gpsimd.dma_start(xT_a[64:E1, 0:256], xta_d[64:E1, 0:256])
        nc.sync.dma_start(xT_b[0:38, 0:256], xtb_d[0:38, 0:256])
        nc.gpsimd.dma_start(xT_b[38:E2, 0:256], xtb_d[38:E2, 0:256])
        nc.sync.dma_start(w_a[0:64, :], w_d[0:64, :])
        nc.gpsimd.dma_start(w_a[64:E1, :], w_d[64:E1, :])
        nc.sync.dma_start(xq_b[0:38, :], xqb_d[0:38, :])
        nc.gpsimd.dma_start(xq_b[38:E2, :], xqb_d[38:E2, :])
        nc.scalar.dma_start(w_b[:], w_d[E1:E, :])
        nc.sync.dma_start(xT_a[0:64, 256:2048], xta_d[0:64, 256:2048])
        nc.gpsimd.dma_start(xT_a[64:E1, 256:2048], xta_d[64:E1, 256:2048])
        nc.sync.dma_start(xT_b[0:38, 256:2048], xtb_d[0:38, 256:2048])
        nc.gpsimd.dma_start(xT_b[38:E2, 256:2048], xtb_d[38:E2, 256:2048])
        nc.gpsimd.dma_start(pad_sb[:], pad_d[:])
        make_identity(nc, identf[:])  # gpsimd, after the urgent triggers
        # tri01[k, q] = 1 if k <= q else 0   (strict lower triangle masked)
        nc.gpsimd.memset(tri01[:], 1.0)
        nc.gpsimd.affine_select(
            out=tri01[:],
            in_=tri01[:],
            compare_op=mybir.AluOpType.is_ge,
            fill=0.0,
            base=0,
            pattern=[[1, 128]],  # iota = -k + q ; keep where >= 0
            channel_multiplier=-1,
        )
        wsb = {
            "wq": (w_a[:, 0:H], w_b[:, 0:H]),
            "wk": (w_a[:, H : 2 * H], w_b[:, H : 2 * H]),
            "wv": (w_a[:, 2 * H : 3 * H], w_b[:, 2 * H : 3 * H]),
        }
        wqa, wqb = wsb["wq"]
        wka, wkb = wsb["wk"]
        wva, wvb = wsb["wv"]

        ones = const.tile([128, NT], bf16)
        nc.vector.memset(ones[:], 1.0)
        trash = const.tile([128, 1], f32)
        # preload the Exp table on ACT while the queue is idle: the implicit
        # ACT_TABLE_LOAD (1.3us) then precedes the dummy, not the first real exp
        dummy = const.tile([128, 1], f32)
        nc.scalar.activation(dummy[:], ones[:, 0:1], Exp)

        QT = big.tile([128, T // 2], bf16)  # own queries only, chunk-contig
        KT = big.tile([128, T], bf16)
        # zero the padding rows (64:128): S matmuls then run at full 128-row
        # contraction occupancy, which keeps the HAM clock governor promoted
        nc.vector.memset(QT[H:128, :], 0.0)
        nc.gpsimd.memset(KT[H:128, 0:2048], 0.0)
        nc.gpsimd.memset(KT[H:128, 2048:T], 0.0)
        vaug = big.tile([128, NT * (H + 1)], bf16)
        vaug_r = vaug[:].rearrange("p (k c) -> p k c", c=H + 1)
        nc.vector.tensor_copy(
            vaug_r[:, :, H : H + 1],
            ones[:].rearrange("p (k o) -> p k o", o=1),
        )
        ybig = big.tile([128, 16 * H], f32)
        ybig_r = ybig[:].rearrange("p (q c) -> p q c", c=H)

        # even 128-col tiles of x^T = this core's query tokens
        xta_r = xT_a[:].rearrange("p (j t) -> p j t", t=256)
        xtb_r = xT_b[:].rearrange("p (j t) -> p j t", t=256)

        # ---- PE warm burst on const data: HAM promotion to 2.4 GHz needs
        # ~4.2us of continuous high-utilization matmuls; this burst covers the
        # DMA wait, and warm_b (iteration 0) bridges through the promotion.
        warm = spool.tile([128, 128], f32, tag="S", name="warm")
        for _ in range(40):
            nc.tensor.matmul(warm[:], identb[:], identb[:], start=True, stop=True)

        # ---- projection units (woven between attention iterations) ----
        def proj_K(k0, n):
            c0, w = k0 * 128, n * 128
            ps = upool.tile([H, w], f32, tag="U", name="psk")
            nc.tensor.matmul(ps[:], wka, xT_a[:, c0 : c0 + w], start=True, stop=False)
            nc.tensor.matmul(ps[:], wkb, xT_b[:, c0 : c0 + w], start=False, stop=True)
            nc.vector.tensor_copy(KT[0:H, c0 : c0 + w], ps[:])

        def proj_V(k0, n=2):
            # V directly in [token, H] layout: the x tile is the matmul
            # stationary, W the moving operand -- no transpose, no staging.
            # One PSUM tile + one cast per pair keeps upool ring pressure low.
            psv = upool.tile([128, n * H], f32, tag="U", name="psv")
            for j in range(n):
                kt = k0 + j
                nc.tensor.matmul(
                    psv[:, j * H : (j + 1) * H],
                    xT_a[:, kt * 128 : (kt + 1) * 128], wva,
                    start=True, stop=False,
                )
                nc.tensor.matmul(
                    psv[:, j * H : (j + 1) * H],
                    xT_b[:, kt * 128 : (kt + 1) * 128], wvb,
                    start=False, stop=True,
                )
            nc.vector.tensor_copy(
                vaug_r[:, k0 : k0 + n, 0:H],
                psv[:].rearrange("p (k c) -> p k c", c=H),
            )

        def proj_Q(a):
            ps = upool.tile([H, 512], f32, tag="U", name="psq")
            if a < 2:
                ra, rb = xq_a[:, a * 512 : (a + 1) * 512], xq_b[:, a * 512 : (a + 1) * 512]
            else:
                ra = xta_r[:, 4 * a : 4 * a + 4, 0:128]
                rb = xtb_r[:, 4 * a : 4 * a + 4, 0:128]
            nc.tensor.matmul(ps[:], wqa, ra, start=True, stop=False)
            nc.tensor.matmul(ps[:], wqb, rb, start=False, stop=True)
            nc.vector.tensor_copy(QT[0:H, a * 512 : (a + 1) * 512], ps[:])

        # prolog: K tiles 0-3 (a lead so in-loop S never waits on a fresh
        # cast) and Q chunks 0+1 which gate S kt=0
        proj_K(0, 2)
        proj_Q(0)
        proj_Q(1)

        # Early units feed pair (0,1); late units (iters 17+) fill the PE
        # slack of the long pair (2,3) off-diagonal phase.
        units = [lambda: proj_K(2, 2), lambda: proj_V(0)]
        for m in range(6):
            units.append(lambda k0=4 + 2 * m: proj_K(k0, 2))  # K 4..15
            units.append(lambda k0=2 + 2 * m: proj_V(k0))  # V 2..13
        units.append(lambda: proj_Q(2))
        units.append(lambda: proj_Q(3))
        units.append(lambda: proj_V(14))
        for m in range(8):
            units.append(lambda k0=16 + 2 * m: proj_K(k0, 2))  # K 16..31
            units.append(lambda k0=16 + 2 * m: proj_V(k0))  # V 16..31
        # units popped per iteration: just-in-time for pair(0,1), spread so no
        # iteration carries more than ~1 unit of extra PE work where possible
        POPS = [2, 1, 1, 1, 1, 2, 2, 1, 1, 2, 1, 2] + [0] * 5 + [1] * 23

        def emit_units(k):
            for _ in range(k):
                if units:
                    units.pop(0)()

        # ---- epilogue: acc^T via PE transpose; out = acc^T[:, :64]/acc^T[:, 64]
        def epilogue(oa, a):
            pf = upool.tile([128, 4 * (H + 1)], f32, tag="U", name="pf")
            pf_r = pf[:].rearrange("p (j c) -> p j c", c=H + 1)
            for j in range(4):
                nc.tensor.transpose(
                    pf_r[:, j, :],
                    oa[:, j * 128 : (j + 1) * 128],
                    identf[0 : H + 1, 0 : H + 1],
                )
            r = rpool.tile([128, 4], f32)
            nc.vector.reciprocal(r[:], pf_r[:, :, H : H + 1])
            for j in range(4):
                nc.vector.tensor_scalar_mul(
                    ybig_r[:, 4 * a + j, :], pf_r[:, j, 0:H], r[:, j : j + 1]
                )
            nc.sync.dma_start(
                y_d[4 * a * 128 : (4 * a + 4) * 128, :].rearrange(
                    "(q p) c -> p q c", p=128
                ),
                ybig_r[:, 4 * a : 4 * a + 4, :],
            )

        epi_q = []  # [countdown, oa, a]: PE transposes deferred 2 iterations

        def finish_half(accT, half, a):
            oa = oapool.tile([H + 1, 512], f32, name="oa")
            nc.vector.tensor_copy(oa[:], accT[:, half * 512 : half * 512 + 512])
            epi_q.append([2, oa, a])

        def emit_av(avs):
            for entry in avs:
                if entry[0] == "pair":
                    # matmul outputs are capped at 512 cols (one PSUM bank):
                    # two mms sharing the vslice stationary
                    _, kt, a0, lo, accT, vslice, P = entry
                    nc.tensor.matmul(
                        accT[:, lo:512], vslice, P[:, lo:512],
                        start=(kt == 0), stop=(kt == 8 * a0 + 7),
                        skip_group_check=True,
                    )
                    nc.tensor.matmul(
                        accT[:, 512:1024], vslice, P[:, 512:1024],
                        start=(kt == 0), stop=False, skip_group_check=True,
                    )
                    if kt == 8 * a0 + 7:
                        finish_half(accT, 0, a0)
                else:
                    _, k2, a, v0, accT, vslice, P, s0, w1 = entry
                    nc.tensor.matmul(
                        accT[:, 512 + v0 * 128 : 1024], vslice, P[:, s0 : s0 + w1],
                        start=False, stop=(k2 == 8 * a + 7), skip_group_check=True,
                    )
                    if k2 == 8 * a + 7:
                        finish_half(accT, 1, a)

        # ---- attention: chunk-pair outer (2 live accumulators side by side
        # in one PSUM tile), k-tiles inner. Software-pipelined: AV of
        # iteration kt issues after S/exp of kt+1.
        pending = None
        it = 0
        for pi, (a0, a1) in enumerate([(0, 1), (2, 3)]):
            accT = accpool.tile([H + 1, 1024], f32, tag="acc", name=f"accT{pi}")
            ext = 8 * a1 + 8
            kt = 0
            while kt < ext:
                am = kt // 8
                u = kt - 8 * am
                v0 = u // 2
                S = spool.tile([128, 1024], f32, tag="S")
                P = ppool.tile([128, 1024], bf16)
                if am <= a0:
                    # one k-tile feeding both accumulators: two 512-col mms
                    # (ISA caps a matmul output at one PSUM bank) sharing the
                    # kslice stationary, one exp over the contiguous range
                    lo = v0 * 128 if a0 == am else 0
                    kslice = KT[:, kt * 128 : (kt + 1) * 128]
                    vslice = vaug[:, kt * (H + 1) : (kt + 1) * (H + 1)]
                    nc.tensor.matmul(
                        S[:, lo:512],
                        kslice,
                        QT[:, a0 * 512 + lo : (a0 + 1) * 512],
                        start=True,
                        stop=True,
                    )
                    nc.tensor.matmul(
                        S[:, 512:1024],
                        kslice,
                        QT[:, a1 * 512 : (a1 + 1) * 512],
                        start=True,
                        stop=True,
                    )
                    nc.scalar.activation(P[:, lo:1024], S[:, lo:1024], Exp, scale=SCALE)

                    def masks(P=P, v0=v0, u=u, diag=(a0 == am)):
                        if diag:
                            blk = P[:, v0 * 128 : v0 * 128 + 128]
                            if u % 2 == 0:
                                nc.gpsimd.tensor_mul(blk, blk, tri01[:])
                            else:
                                nc.vector.tensor_scalar_mul(blk, blk, pad_sb[:])

                    avs = [("pair", kt, a0, lo, accT, vslice, P)]
                    kt += 1
                else:
                    # two k-tiles, one live accumulator (tail of the chunk):
                    # compact layout -- no dead zone between the halves
                    w1 = 512 - v0 * 128
                    avs = []
                    for idx in range(2):
                        k2 = kt + idx
                        s0 = v0 * 128 if idx == 0 else 512
                        nc.tensor.matmul(
                            S[:, s0 : s0 + w1],
                            KT[:, k2 * 128 : (k2 + 1) * 128],
                            QT[:, a1 * 512 + v0 * 128 : (a1 + 1) * 512],
                            start=True,
                            stop=True,
                        )
                        avs.append(
                            ("single", k2, a1, v0, accT,
                             vaug[:, k2 * (H + 1) : (k2 + 1) * (H + 1)], P, s0, w1)
                        )
                    nc.scalar.activation(
                        P[:, v0 * 128 : 512 + w1], S[:, v0 * 128 : 512 + w1],
                        Exp, scale=SCALE,
                    )

                    # u even tile gets the triangle, u+1 (odd) the pad mask
                    def masks(P=P, v0=v0):
                        blk0 = P[:, v0 * 128 : v0 * 128 + 128]
                        nc.gpsimd.tensor_mul(blk0, blk0, tri01[:])
                        blk1 = P[:, 512:640]
                        nc.vector.tensor_scalar_mul(blk1, blk1, pad_sb[:])

                    kt += 2
                # units first so their DVE casts aren't head-blocked behind
                # the exp-gated mask ops on the in-order vector queue
                emit_units(POPS[it] if it < len(POPS) else 1)
                masks()
                # second half of x, triggered mid-loop so these bulk
                # transfers don't steal DMA bandwidth from the prolog-critical
                # loads. The scheduler hoists dependency-free DMAs, so give
                # each a real WAR hazard: a 1-element read of its destination
                # gated on this iteration's P tile (i.e. on exp_it).
                if it in (1, 3, 5):
                    dst, src_d, p0 = {
                        1: (xT_a[0:64, 2048:T], xta_d[0:64, 2048:T], 0),
                        3: (xT_a[64:E1, 2048:T], xta_d[64:E1, 2048:T], 64),
                        5: (xT_b[:, 2048:T], xtb_d[:, 2048:T], 0),
                    }[it]
                    nc.vector.tensor_mul(
                        trash[p0 : p0 + 1, 0:1], dst[0:1, 0:1], P[p0 : p0 + 1, 0:1]
                    )
                    nc.gpsimd.dma_start(dst, src_d)
                if pending is not None:
                    emit_av(pending)
                pending = avs
                for e in list(epi_q):
                    e[0] -= 1
                    if e[0] <= 0:
                        epi_q.remove(e)
                        epilogue(e[1], e[2])
                it += 1
        emit_av(pending)  # acc3 stop + oa copy; its epilogue drains below
        # keep the PE (and the HAM clock) busy while the DVE copies oa3
        warm2 = spool.tile([128, 128], f32, tag="S", name="warm2")
        for _ in range(36):
            nc.tensor.matmul(warm2[:], identb[:], identb[:], start=True, stop=True)
        for e in epi_q:
            epilogue(e[1], e[2])

    nc.compile()
    return nc


def _get_nc():
    if "nc" not in _CACHE:
        _CACHE["nc"] = _build_nc()
    return _CACHE["nc"]


_PAIR_SWAP = np.arange(NT).reshape(-1, 2)[:, ::-1].reshape(-1)  # [1,0,3,2,...]


def _make_in_maps(x, Wq, Wk, Wv):
    from ml_dtypes import bfloat16

    x = np.asarray(x, dtype=np.float32)
    Wall = np.ascontiguousarray(
        np.concatenate(
            [np.asarray(W, dtype=np.float32) for W in (Wq, Wk, Wv)], axis=1
        ).astype(bfloat16)
    )
    assert x.shape == (B, T, E)
    in_maps = []
    for c in range(NCORES):
        b, s = c // 2, c % 2
        xb = x[b]
        if s == 1:
            xb = xb.reshape(NT, 128, E)[_PAIR_SWAP].reshape(T, E)
        xt = np.ascontiguousarray(xb.T.astype(bfloat16))  # [E, T]
        # packed even 128-col tiles of cols 0:2048 = Q chunks 0+1
        xq = np.ascontiguousarray(
            xt[:, 0:2048].reshape(E, 8, 256)[:, :, 0:128].reshape(E, 1024)
        )
        in_maps.append(
            {
                "xta": np.ascontiguousarray(xt[0:E1]),
                "xtb": np.ascontiguousarray(xt[E1:E]),
                "xqa": np.ascontiguousarray(xq[0:E1]),
                "xqb": np.ascontiguousarray(xq[E1:E]),
                "w_all": Wall,
                "pad01": np.full((128, 1), 0.0 if s == 0 else 1.0, np.float32),
            }
        )
    return in_maps


def _gather(results):
    y = np.empty((B, T, H), dtype=np.float32)
    for c in range(NCORES):
        b, s = c // 2, c % 2
        yl = np.asarray(results[c]["y"]).reshape(T // 256, 128, H)
        yv = y[b].reshape(NT, 128, H)
        yv[2 * np.arange(T // 256) + s] = yl
    return y


def kernel(x, Wq, Wk, Wv, mask=True, **_ignored):
    assert bool(mask), "kernel compiled for causal (mask=True)"
    nc = _get_nc()
    from concourse import bass_utils

    in_maps = _make_in_maps(x, Wq, Wk, Wv)
    res = bass_utils.run_bass_kernel_spmd(nc, in_maps, list(range(NCORES)))
    _CACHE["last_result"] = res
    return _gather(res.results)


if __name__ == "__main__":
    # smoke test with random data
    rng = np.random.default_rng(0)
    x = rng.standard_normal((B, T, E), dtype=np.float32)
    s = 1.0 / np.sqrt(E)
    Wq = (rng.standard_normal((E, H)) * s).astype(np.float32)
    Wk = (rng.standard_normal((E, H)) * s).astype(np.float32)
    Wv = (rng.standard_normal((E, H)) * s).astype(np.float32)
    out = kernel(x, Wq, Wk, Wv, True)
    print("out", out.shape, out.dtype, float(np.abs(out).max()))
